# revision 26
# baseline (speedup 1.0000x reference)
"""AutoCorrelation (Autoformer-style) Bass kernel for one TRN2 chip (8 NeuronCores).

Math: the reference computes, per (b, h):
    corr = irfft(rfft(q, axis=-1) * conj(rfft(k, axis=-1)), n=L)   # [L, L]
    weights = softmax(corr - mean_h(corr), axis=-1)
    Vt = v @ weights                                                # [d, L]
The rfft runs over the d=64 channel axis and the irfft zero-pads 33 bins to
L=2048, so corr[s, :] is a rank-<=66 function of l; the DC term is constant
over l and cancels in softmax.  Collapsing the spectral products
(re*re + im*im -> cos row, im*re - re*im -> -sin row) leaves 64 coefficient
rows: the logits are an exact K=64 matmul against a fixed cos/sin basis and
no [L, L] tensor ever exists in DRAM.

Sharding: head h -> core h (both batches per core).  The 64 x 2048
coefficient matrices (0.5 MB/core) are computed on the HOST in fp32 --
they are <1% of the FLOPs, and the head-mean that couples cores becomes a
trivial host reduction (an on-device AllReduce costs 55-60 us of fixed
latency on this platform).  The device runs ONE NEFF: the [L, L]-shaped
softmax + delay aggregation, which is >99% of the compute.

Device kernel structure per (b, s-chunk of 128 rows):
  - logits: two row-packed K=64 PE matmul pairs (cd duplicated to partition
    halves 0/64 of both cd and basis) -> two [128, 1024] fp32 PSUM tiles
  - exp: ScalarE table-exp on tile 0, VectorE custom DVE op EXP8_ANT
    (exp(x) ~= (c0 + x(c1 + x c2))^8, valid for |logit| <= ~1.7) on tile 1,
    both writing bf16 weights to SBUF with fused free-dim accumulation for
    the softmax denominator
  - aggregation: column-packed K=128 matmul pairs accumulating Vt into a
    single [128, 1024] fp32 PSUM tile (partitions 0-63 = l 0:1024,
    64-127 = l 1024:2048); the per-row 1/sum folds into the tiny v tile
  - normalization chores (denominator add, reciprocal, v-row scaling) are
    batched 4 s-chunks at a time so GpSimd's ~1us/instruction overhead and
    DVE's per-op overhead amortize; the aggregation lags 6 chunks behind.
PSUM: 3 rotating [128,1024] logit tiles (6 banks) + Vt accumulator (2).
A short burst of dummy matmuls during the DMA preamble warms the PE HAM
clock gate so real matmuls run at 2.4 GHz from the start.
"""
import sys
from operator import add as _op_add

sys.path.insert(0, "/opt/trn_rl_repo")

import numpy as np
import ml_dtypes

from concourse import bass, bacc, mybir, tile
from concourse import dve_ops
from concourse.dve_spec import Spec, Src0, C0, C1, C2, Zero, sq, lower
from concourse.dve_uop import DveOpSpec
from concourse.bass_utils import run_bass_kernel_spmd

B, L, E, H, D = 2, 2048, 512, 8, 64
NF = 32          # frequencies 1..32 of the 64-point rfft (DC dropped)
NCOMP = 4 * NF   # 128 raw product rows
NCC = 2 * NF     # 64 compressed coefficient rows (cos, sin)
NCORES = 8
SC = L // 128    # 16 s-chunks of 128 rows
GRP = 4          # s-chunks per normalization batch
ACC_LAG = 6      # aggregation runs this many s-chunks behind the logits
WARM_MM = 13     # dummy matmuls to warm the PE clock gate during DMA
BF16 = mybir.dt.bfloat16
F32 = mybir.dt.float32

# minimax quadratic p(z) for e^z on z = x/8, |x| <= 1.68; exp(x) ~= p(x)^8
EXP_C = (0.99970171, 0.12580122, 0.00795605)

TRACE = False
LAST_RESULT = None
LAST_RESULT_A = None

_COMPILED = None
_EXP_OP = None


def _register_exp_op():
    global _EXP_OP
    if _EXP_OP is not None:
        return _EXP_OP
    for o in dve_ops.OPS:
        if o.name == "EXP8_ANT":
            _EXP_OP = o
            return o

    body = sq(sq(sq(C0 + Src0 * (C1 + Src0 * C2))))

    def _ref(in0, in1, c0, c1, c2):
        x = in0.astype(np.float32)
        b = (((c0 + x * (c1 + x * c2)) ** 8)).astype(np.float32)
        return b, b.reshape(b.shape[0], -1).sum(axis=-1, keepdims=True)

    spec = Spec(body=body, accum=_op_add, accum_init=Zero, reference=_ref)
    opcode = dve_ops._CUSTOM_DVE_ROW_BASE + len(dve_ops.OPS)
    dve_ops._SUB_OPCODE_FOR_NAME["EXP8_ANT"] = opcode
    shas = {}
    for ver in ("v3", "v4"):
        shas[ver] = DveOpSpec(
            name="EXP8_ANT", opcode=opcode, uops=lower(spec, ver=ver), rd1_en=False
        ).sha(ver)
    op = dve_ops.DveOp("EXP8_ANT", spec, subdim=False, uops_sha=shas)
    dve_ops.OPS.append(op)
    dve_ops.CUSTOM_DVE_SPECS[op.name] = spec
    _EXP_OP = op
    return op


def _constants_f32():
    c = np.arange(D)
    f = np.arange(1, NF + 1)
    ang = 2 * np.pi * np.outer(c, f) / D
    fcos = np.cos(ang)       # Re X_f   = sum_c q_c cos
    fsin = -np.sin(ang)      # Im X_f   = -sum_c q_c sin
    w = 2.0 / L              # irfft weight for interior bins
    fx = np.concatenate([fcos * w, fsin * w, fsin * w, fcos * w], axis=1)  # [64, 128]
    fy = np.concatenate([fcos, fsin, fcos, fsin], axis=1)                  # [64, 128]
    t = np.arange(L)
    angt = 2 * np.pi * np.outer(f, t) / L
    cosb, sinb = np.cos(angt), np.sin(angt)
    basis64 = np.concatenate([cosb, -sinb], axis=0)                        # [64, 2048]
    # compression: Ccs[0:32] = P[0:32] + P[32:64]  (re*re + im*im -> cos)
    #              Ccs[32:64] = P[64:96] - P[96:128] (im*re - re*im -> -sin)
    mcomp = np.zeros((NCOMP, NCC), np.float32)
    for m in range(32):
        mcomp[m, m] = 1.0
        mcomp[m + 32, m] = 1.0
        mcomp[m + 64, m + 32] = 1.0
        mcomp[m + 96, m + 32] = -1.0
    return (fx.astype(np.float32), fy.astype(np.float32),
            basis64.astype(np.float32), mcomp)


def _build():
    exp_op = _register_exp_op()
    nc = bacc.Bacc("TRN2", target_bir_lowering=False, debug=False, num_devices=NCORES)

    cd_d = nc.dram_tensor("cd", [B, NCC, L], BF16, kind="ExternalInput")
    v_d = nc.dram_tensor("v", [B, 128, SC, D], BF16, kind="ExternalInput")
    basis_d = nc.dram_tensor("basis", [NCC, L], BF16, kind="ExternalInput")
    out_d = nc.dram_tensor("out", [B, D, L], F32, kind="ExternalOutput")

    with tile.TileContext(nc) as tc:
        with (
            tc.tile_pool(name="consts", bufs=1) as consts,
            tc.tile_pool(name="vv", bufs=2) as v_pool,
            tc.tile_pool(name="cd", bufs=2) as cd_pool,
            tc.tile_pool(name="wts", bufs=24) as w_pool,
            tc.tile_pool(name="small", bufs=12) as s_pool,
            tc.tile_pool(name="outp", bufs=2) as out_pool,
            tc.tile_pool(name="ps_log", bufs=3, space="PSUM") as ps_log,
            tc.tile_pool(name="ps_vt", bufs=1, space="PSUM") as ps_vt,
        ):
            # ---- DMA preamble: dedup duplicated partition halves, spread
            # across queues, order so b=0's operands land first. ----
            basis_sb = consts.tile([NCOMP, L], BF16)
            nc.sync.dma_start(out=basis_sb[0:NCC, :], in_=basis_d[:])
            nc.sync.dma_start(out=basis_sb[NCC:NCOMP, :], in_=basis_d[:])
            cd_sbs = []
            for b in range(B):
                cdd = cd_pool.tile([2 * NCC, L], BF16, tag=f"cd{b}")
                eng = nc.scalar
                eng.dma_start(out=cdd[0:NCC, :], in_=cd_d[b])
                eng.dma_start(out=cdd[NCC:2 * NCC, :], in_=cd_d[b])
                cd_sbs.append(cdd)
            v_sbs = []
            for b in range(B):
                v_sb = v_pool.tile([128, SC, D], BF16, tag=f"v{b}")
                nc.gpsimd.dma_start(out=v_sb[:], in_=v_d[b])
                v_sbs.append(v_sb)

            # ---- PE warmup: junk matmuls (no DMA deps) flip the HAM clock
            # gate to 2.4 GHz while the inputs stream in. ----
            junk = consts.tile([128, 512], BF16)
            nc.vector.memset(junk[:], 0.0)
            warm_ps = ps_log.tile([128, 1024], F32, tag="log")
            for _ in range(WARM_MM):
                nc.tensor.matmul(
                    warm_ps[:, 0:512], junk[:, 0:128], junk[:, 0:512],
                    start=True, stop=True,
                )

            carry_work = []
            for b in range(B):
                cdd = cd_sbs[b]
                v_sb = v_sbs[b]
                vt_ps = ps_vt.tile([128, 1024], F32, tag="vt")

                wts = {}
                vts_tiles = {}
                sig_tiles = {}

                def emit_acc(sc, wts=wts, vts_tiles=vts_tiles,
                             vt_ps=vt_ps):
                    wt0, wt1 = wts.pop(sc)
                    g, j = divmod(sc, GRP)
                    vts = vts_tiles[g]
                    lhs = vts[:, j, :]
                    for q in range(2):
                        # column-packed pair: l-half 0 on PE cols 0-63,
                        # l-half 1 on cols 64-127, concurrent
                        nc.tensor.matmul(
                            vt_ps[0:D, q * 512:(q + 1) * 512],
                            lhs, wt0[:, q * 512:(q + 1) * 512],
                            start=(sc == 0), stop=(sc == SC - 1),
                        )
                        nc.tensor.matmul(
                            vt_ps[D:2 * D, q * 512:(q + 1) * 512],
                            lhs, wt1[:, q * 512:(q + 1) * 512],
                            start=(sc == 0), stop=(sc == SC - 1),
                        )
                    if j == GRP - 1:
                        vts_tiles.pop(g)

                def emit_group(g):
                    # denominators for s-chunks 4g..4g+3 are complete:
                    # Z = sigS + sigV, rcp = 1/Z, vts = v * rcp -- one
                    # batched instruction each.
                    sig = sig_tiles.pop(g)
                    zs = s_pool.tile([128, GRP], F32, tag="zs", name=f"zs{b}_{g}")
                    nc.gpsimd.tensor_add(zs[:], sig[:, 0:GRP], sig[:, GRP:2 * GRP])
                    rcp = s_pool.tile([128, GRP], F32, tag="rcp", name=f"rcp{b}_{g}")
                    nc.vector.reciprocal_approx_fast(rcp[:], zs[:])
                    vts = s_pool.tile([128, GRP, D], BF16, tag="vts", name=f"vts{b}_{g}")
                    rcp_b = rcp[:].broadcast_to([128, GRP, D])
                    nc.gpsimd.tensor_mul(vts[:], v_sb[:, GRP * g:GRP * (g + 1), :],
                                         rcp_b)
                    vts_tiles[g] = vts

                for sc in range(SC):
                    off = sc * 128
                    cdt = cdd[0:NCC, off:off + 128]
                    cdb = cdd[NCC:2 * NCC, off:off + 128]
                    # aggregation first: it is always ready (lags 5
                    # chunks), so the in-order PE never head-of-line blocks
                    # on a logit-tile wait while ready work sits behind it
                    if sc >= ACC_LAG:
                        emit_acc(sc - ACC_LAG)
                    # drain the previous batch's carried-over aggregation
                    # flush + Vt drain into our early iterations, so the PE
                    # and DMA queues never go quiet at the batch boundary
                    for _ in range(2):
                        if carry_work:
                            carry_work.pop(0)()
                    lg0 = ps_log.tile([128, 1024], F32, tag="log")
                    lg1 = ps_log.tile([128, 1024], F32, tag="log")
                    for q in range(2):
                        # row-packed pair: l-half 0 on PE rows 0-63,
                        # l-half 1 on rows 64-127, concurrent
                        nc.tensor.matmul(
                            lg0[:, q * 512:(q + 1) * 512], cdt,
                            basis_sb[0:NCC, q * 512:(q + 1) * 512],
                            start=True, stop=True,
                        )
                        nc.tensor.matmul(
                            lg1[:, q * 512:(q + 1) * 512], cdb,
                            basis_sb[NCC:2 * NCC, 1024 + q * 512: 1024 + (q + 1) * 512],
                            start=True, stop=True,
                        )
                    if b == 0 and sc < 4:
                        # keep the PE busy while the first exps drain the
                        # logit tiles -- a >3.4us PE-idle gap here drops the
                        # HAM clock gate back to 1.2 GHz for ~10us
                        for _ in range(5):
                            nc.tensor.matmul(
                                vt_ps[:, 0:512], junk[:, 0:128],
                                junk[:, 0:512], start=True, stop=True,
                                skip_group_check=True,
                            )
                    g, j = divmod(sc, GRP)
                    if j == 0:
                        sig_tiles[g] = s_pool.tile([128, 2 * GRP], F32, tag="sig",
                                                    name=f"sig{b}_{g}")
                    sig = sig_tiles[g]
                    wt0 = w_pool.tile([128, 1024], BF16, tag="wt")
                    wt1 = w_pool.tile([128, 1024], BF16, tag="wt")
                    # alternate which engine exps which tile: with 3 rotating
                    # logit tiles and 2 consumers, a fixed assignment couples
                    # S's input availability to V's exp two chunks prior (and
                    # vice versa) -- a period-2 limit cycle that settles at
                    # 1.87us/chunk instead of 1.19us.  Parity-swapping the
                    # consumers breaks the resonance.
                    s_in, s_out = (lg0, wt0) if sc % 2 == 0 else (lg1, wt1)
                    v_in, v_out = (lg1, wt1) if sc % 2 == 0 else (lg0, wt0)
                    nc.scalar.activation(
                        s_out[:], s_in[:], mybir.ActivationFunctionType.Exp,
                        accum_out=sig[:, j:j + 1],
                    )
                    nc.vector._custom_dve(
                        exp_op, out=v_out[:], in0=v_in[:],
                        s0=EXP_C[0], s1=EXP_C[1], imm2=EXP_C[2],
                        accum_out=sig[:, GRP + j:GRP + j + 1],
                    )
                    wts[sc] = (wt0, wt1)
                    if j == GRP - 1:
                        emit_group(g)

                def drain_b(b=b, vt_ps=vt_ps):
                    # drain Vt: split across ScalarE/VectorE (different PSUM
                    # banks -> parallel), then store the l-halves on the two
                    # idle DMA queues
                    out_sb = out_pool.tile([128, 1024], F32, tag="out",
                                           name=f"out{b}")
                    nc.scalar.copy(out_sb[:, 0:512], vt_ps[:, 0:512])
                    nc.vector.tensor_copy(out_sb[:, 512:1024],
                                          vt_ps[:, 512:1024])
                    nc.sync.dma_start(out=out_d[b][:, 0:1024],
                                      in_=out_sb[0:D, :])
                    nc.gpsimd.dma_start(out=out_d[b][:, 1024:2048],
                                        in_=out_sb[D:2 * D, :])

                tail = [(lambda sc=sc, f=emit_acc: f(sc))
                        for sc in range(SC - ACC_LAG, SC)] + [drain_b]
                if b < B - 1:
                    carry_work = tail
                else:
                    for fn in tail:
                        fn()

    nc.compile()
    return nc


def _get_compiled():
    global _COMPILED
    if _COMPILED is None:
        _COMPILED = _build()
    return _COMPILED


def kernel(queries, keys, values):
    global LAST_RESULT
    queries = np.asarray(queries, dtype=np.float32)
    keys = np.asarray(keys, dtype=np.float32)
    values = np.asarray(values, dtype=np.float32)
    bf = ml_dtypes.bfloat16

    # ---- Host coefficient phase (fp32): spectra, cross products,
    # compression, head-mean subtraction.  <1% of total FLOPs. ----
    fx, fy, basis64, mcomp = _constants_f32()
    QT = np.ascontiguousarray(
        queries.reshape(B, L, H, D).transpose(2, 0, 3, 1))   # [H,B,D,L]
    KT = np.ascontiguousarray(
        keys.reshape(B, L, H, D).transpose(2, 0, 3, 1))
    xt = np.matmul(fx.T[None, None], QT)                     # [H,B,128,L]
    yt = np.matmul(fy.T[None, None], KT)
    cf = xt * yt
    ccs = np.matmul(mcomp.T[None, None], cf)                 # [H,B,64,L]
    cd = (ccs - ccs.mean(axis=0, keepdims=True)).astype(bf)  # [H,B,64,L]

    v_swz = np.ascontiguousarray(
        values.reshape(B, SC, 128, H, D).transpose(3, 0, 2, 1, 4)
    ).astype(bf)                                             # [H,B,128,SC,D]
    basis_bf = basis64.astype(bf)

    in_maps = [
        {"cd": np.ascontiguousarray(cd[i]),
         "v": v_swz[i],
         "basis": basis_bf}
        for i in range(NCORES)
    ]

    kw = {"trace_cores": list(range(NCORES))} if TRACE else {}
    cores = list(range(NCORES))
    nc = _get_compiled()
    res = run_bass_kernel_spmd(nc, in_maps, core_ids=cores, trace=TRACE, **kw)
    LAST_RESULT = res

    vt_full = np.stack([res.results[i]["out"] for i in range(NCORES)], axis=1)
    # reference: out = transpose(Vt[B,H,d,L], (0,2,1,3)).reshape(B, L, H*d)
    return np.ascontiguousarray(
        vt_full.transpose(0, 2, 1, 3).reshape(B, L, E)
    ).astype(np.float32)


# revision 28
# speedup vs baseline: 1.0292x; 1.0292x over previous
"""AutoCorrelation (Autoformer-style) Bass kernel for one TRN2 chip (8 NeuronCores).

Math: the reference computes, per (b, h):
    corr = irfft(rfft(q, axis=-1) * conj(rfft(k, axis=-1)), n=L)   # [L, L]
    weights = softmax(corr - mean_h(corr), axis=-1)
    Vt = v @ weights                                                # [d, L]
The rfft runs over the d=64 channel axis and the irfft zero-pads 33 bins to
L=2048, so corr[s, :] is a rank-<=66 function of l; the DC term is constant
over l and cancels in softmax.  Collapsing the spectral products
(re*re + im*im -> cos row, im*re - re*im -> -sin row) leaves 64 coefficient
rows: the logits are an exact K=64 matmul against a fixed cos/sin basis and
no [L, L] tensor ever exists in DRAM.

Sharding: head h -> core h (both batches per core).  The 64 x 2048
coefficient matrices (0.5 MB/core) are computed on the HOST in fp32 --
they are <1% of the FLOPs, and the head-mean that couples cores becomes a
trivial host reduction (an on-device AllReduce costs 55-60 us of fixed
latency on this platform).  The device runs ONE NEFF: the [L, L]-shaped
softmax + delay aggregation, which is >99% of the compute.

Device kernel structure per (b, s-chunk of 128 rows):
  - logits: two row-packed K=64 PE matmul pairs (cd duplicated to partition
    halves 0/64 of both cd and basis) -> two [128, 1024] fp32 PSUM tiles
  - exp: ScalarE table-exp on tile 0, VectorE custom DVE op EXP8_ANT
    (exp(x) ~= (c0 + x(c1 + x c2))^8, valid for |logit| <= ~1.7) on tile 1,
    both writing bf16 weights to SBUF with fused free-dim accumulation for
    the softmax denominator
  - aggregation: column-packed K=128 matmul pairs accumulating Vt into a
    single [128, 1024] fp32 PSUM tile (partitions 0-63 = l 0:1024,
    64-127 = l 1024:2048); the per-row 1/sum folds into the tiny v tile
  - normalization chores (denominator add, reciprocal, v-row scaling) are
    batched 4 s-chunks at a time so GpSimd's ~1us/instruction overhead and
    DVE's per-op overhead amortize; the aggregation lags 6 chunks behind.
PSUM: 3 rotating [128,1024] logit tiles (6 banks) + Vt accumulator (2).
A short burst of dummy matmuls during the DMA preamble warms the PE HAM
clock gate so real matmuls run at 2.4 GHz from the start.
"""
import sys
from operator import add as _op_add

sys.path.insert(0, "/opt/trn_rl_repo")

import numpy as np
import ml_dtypes

from concourse import bass, bacc, mybir, tile
from concourse import dve_ops
from concourse.dve_spec import Spec, Src0, C0, C1, C2, Zero, sq, lower
from concourse.dve_uop import DveOpSpec
from concourse.bass_utils import run_bass_kernel_spmd

B, L, E, H, D = 2, 2048, 512, 8, 64
NF = 32          # frequencies 1..32 of the 64-point rfft (DC dropped)
NCOMP = 4 * NF   # 128 raw product rows
NCC = 2 * NF     # 64 compressed coefficient rows (cos, sin)
NCORES = 8
SC = L // 128    # 16 s-chunks of 128 rows
GRP = 4          # s-chunks per normalization batch
ACC_LAG = 6      # aggregation runs this many s-chunks behind the logits
WARM_MM = 13     # dummy matmuls to warm the PE clock gate during DMA
BF16 = mybir.dt.bfloat16
F32 = mybir.dt.float32

# minimax quadratic p(z) for e^z on z = x/8, |x| <= 1.68; exp(x) ~= p(x)^8
EXP_C = (0.99970171, 0.12580122, 0.00795605)

TRACE = False
LAST_RESULT = None
LAST_RESULT_A = None

_COMPILED = None
_EXP_OP = None


def _register_exp_op():
    global _EXP_OP
    if _EXP_OP is not None:
        return _EXP_OP
    for o in dve_ops.OPS:
        if o.name == "EXP8_ANT":
            _EXP_OP = o
            return o

    body = sq(sq(sq(C0 + Src0 * (C1 + Src0 * C2))))

    def _ref(in0, in1, c0, c1, c2):
        x = in0.astype(np.float32)
        b = (((c0 + x * (c1 + x * c2)) ** 8)).astype(np.float32)
        return b, b.reshape(b.shape[0], -1).sum(axis=-1, keepdims=True)

    spec = Spec(body=body, accum=_op_add, accum_init=Zero, reference=_ref)
    opcode = dve_ops._CUSTOM_DVE_ROW_BASE + len(dve_ops.OPS)
    dve_ops._SUB_OPCODE_FOR_NAME["EXP8_ANT"] = opcode
    shas = {}
    for ver in ("v3", "v4"):
        shas[ver] = DveOpSpec(
            name="EXP8_ANT", opcode=opcode, uops=lower(spec, ver=ver), rd1_en=False
        ).sha(ver)
    op = dve_ops.DveOp("EXP8_ANT", spec, subdim=False, uops_sha=shas)
    dve_ops.OPS.append(op)
    dve_ops.CUSTOM_DVE_SPECS[op.name] = spec
    _EXP_OP = op
    return op


def _constants_f32():
    c = np.arange(D)
    f = np.arange(1, NF + 1)
    ang = 2 * np.pi * np.outer(c, f) / D
    fcos = np.cos(ang)       # Re X_f   = sum_c q_c cos
    fsin = -np.sin(ang)      # Im X_f   = -sum_c q_c sin
    w = 2.0 / L              # irfft weight for interior bins
    fx = np.concatenate([fcos * w, fsin * w, fsin * w, fcos * w], axis=1)  # [64, 128]
    fy = np.concatenate([fcos, fsin, fcos, fsin], axis=1)                  # [64, 128]
    t = np.arange(L)
    angt = 2 * np.pi * np.outer(f, t) / L
    cosb, sinb = np.cos(angt), np.sin(angt)
    basis64 = np.concatenate([cosb, -sinb], axis=0)                        # [64, 2048]
    # compression: Ccs[0:32] = P[0:32] + P[32:64]  (re*re + im*im -> cos)
    #              Ccs[32:64] = P[64:96] - P[96:128] (im*re - re*im -> -sin)
    mcomp = np.zeros((NCOMP, NCC), np.float32)
    for m in range(32):
        mcomp[m, m] = 1.0
        mcomp[m + 32, m] = 1.0
        mcomp[m + 64, m + 32] = 1.0
        mcomp[m + 96, m + 32] = -1.0
    return (fx.astype(np.float32), fy.astype(np.float32),
            basis64.astype(np.float32), mcomp)


def _build():
    exp_op = _register_exp_op()
    nc = bacc.Bacc("TRN2", target_bir_lowering=False, debug=False, num_devices=NCORES)

    cd_d = nc.dram_tensor("cd", [B, NCC, L], BF16, kind="ExternalInput")
    v_d = nc.dram_tensor("v", [B, 128, SC, D], BF16, kind="ExternalInput")
    basis_d = nc.dram_tensor("basis", [NCC, L], BF16, kind="ExternalInput")
    out_d = nc.dram_tensor("out", [B, D, L], F32, kind="ExternalOutput")

    with tile.TileContext(nc) as tc:
        with (
            tc.tile_pool(name="consts", bufs=1) as consts,
            tc.tile_pool(name="vv", bufs=2) as v_pool,
            tc.tile_pool(name="cd", bufs=2) as cd_pool,
            tc.tile_pool(name="wts", bufs=24) as w_pool,
            tc.tile_pool(name="small", bufs=12) as s_pool,
            tc.tile_pool(name="outp", bufs=2) as out_pool,
            tc.tile_pool(name="ps_log", bufs=3, space="PSUM") as ps_log,
            tc.tile_pool(name="ps_vt", bufs=1, space="PSUM") as ps_vt,
        ):
            # ---- DMA preamble: dedup duplicated partition halves, spread
            # across queues, order so b=0's operands land first. ----
            basis_sb = consts.tile([NCOMP, L], BF16)
            nc.sync.dma_start(out=basis_sb[0:NCC, :], in_=basis_d[:])
            nc.sync.dma_start(out=basis_sb[NCC:NCOMP, :], in_=basis_d[:])
            cd_sbs = []
            for b in range(B):
                cdd = cd_pool.tile([2 * NCC, L], BF16, tag=f"cd{b}")
                eng = nc.scalar
                eng.dma_start(out=cdd[0:NCC, :], in_=cd_d[b])
                eng.dma_start(out=cdd[NCC:2 * NCC, :], in_=cd_d[b])
                cd_sbs.append(cdd)
            v_sbs = []
            for b in range(B):
                v_sb = v_pool.tile([128, SC, D], BF16, tag=f"v{b}")
                nc.gpsimd.dma_start(out=v_sb[:], in_=v_d[b])
                v_sbs.append(v_sb)

            # ---- PE warmup: junk matmuls (no DMA deps) flip the HAM clock
            # gate to 2.4 GHz while the inputs stream in. ----
            junk = consts.tile([128, 512], BF16)
            nc.vector.memset(junk[:], 0.0)
            warm_ps = ps_log.tile([128, 1024], F32, tag="log")
            for _ in range(WARM_MM):
                nc.tensor.matmul(
                    warm_ps[:, 0:512], junk[:, 0:128], junk[:, 0:512],
                    start=True, stop=True,
                )

            carry_work = []
            for b in range(B):
                cdd = cd_sbs[b]
                v_sb = v_sbs[b]
                vt_ps = ps_vt.tile([128, 1024], F32, tag="vt")

                wts = {}
                vts_tiles = {}
                sig_tiles = {}

                def emit_acc(sc, wts=wts, vts_tiles=vts_tiles,
                             vt_ps=vt_ps):
                    wt0, wt1 = wts.pop(sc)
                    g, j = divmod(sc, GRP)
                    vts = vts_tiles[g]
                    lhs = vts[:, j, :]
                    for q in range(2):
                        # column-packed pair: l-half 0 on PE cols 0-63,
                        # l-half 1 on cols 64-127, concurrent
                        nc.tensor.matmul(
                            vt_ps[0:D, q * 512:(q + 1) * 512],
                            lhs, wt0[:, q * 512:(q + 1) * 512],
                            start=(sc == 0), stop=(sc == SC - 1),
                        )
                        nc.tensor.matmul(
                            vt_ps[D:2 * D, q * 512:(q + 1) * 512],
                            lhs, wt1[:, q * 512:(q + 1) * 512],
                            start=(sc == 0), stop=(sc == SC - 1),
                        )
                    if j == GRP - 1:
                        vts_tiles.pop(g)

                def emit_group(g):
                    # denominators for s-chunks 4g..4g+3 are complete:
                    # Z = sigS + sigV, rcp = 1/Z, vts = v * rcp -- one
                    # batched instruction each.
                    sig = sig_tiles.pop(g)
                    zs = s_pool.tile([128, GRP], F32, tag="zs", name=f"zs{b}_{g}")
                    nc.gpsimd.tensor_add(zs[:], sig[:, 0:GRP], sig[:, GRP:2 * GRP])
                    rcp = s_pool.tile([128, GRP], F32, tag="rcp", name=f"rcp{b}_{g}")
                    nc.vector.reciprocal_approx_fast(rcp[:], zs[:])
                    vts = s_pool.tile([128, GRP, D], BF16, tag="vts", name=f"vts{b}_{g}")
                    rcp_b = rcp[:].broadcast_to([128, GRP, D])
                    nc.gpsimd.tensor_mul(vts[:], v_sb[:, GRP * g:GRP * (g + 1), :],
                                         rcp_b)
                    vts_tiles[g] = vts

                for sc in range(SC):
                    off = sc * 128
                    cdt = cdd[0:NCC, off:off + 128]
                    cdb = cdd[NCC:2 * NCC, off:off + 128]
                    # aggregation first: it is always ready (lags 5
                    # chunks), so the in-order PE never head-of-line blocks
                    # on a logit-tile wait while ready work sits behind it
                    if sc >= ACC_LAG:
                        emit_acc(sc - ACC_LAG)
                    # drain the previous batch's carried-over aggregation
                    # flush + Vt drain into our early iterations, so the PE
                    # and DMA queues never go quiet at the batch boundary
                    for _ in range(2):
                        if carry_work:
                            carry_work.pop(0)()
                    # allocate lg1 first: with the 3-buffer rotation the
                    # FIRST allocation reuses the tile freed longest ago.
                    # lg1 feeds VectorE (1.31us/tile), lg0 feeds ScalarE
                    # (1.25us) -- the slower consumer should get the
                    # longer-lag tile, not the other way around.
                    lg1 = ps_log.tile([128, 1024], F32, tag="log",
                                      name="lg1")
                    lg0 = ps_log.tile([128, 1024], F32, tag="log",
                                      name="lg0")
                    for q in range(2):
                        # row-packed pair: l-half 0 on PE rows 0-63,
                        # l-half 1 on rows 64-127, concurrent
                        nc.tensor.matmul(
                            lg0[:, q * 512:(q + 1) * 512], cdt,
                            basis_sb[0:NCC, q * 512:(q + 1) * 512],
                            start=True, stop=True,
                        )
                        nc.tensor.matmul(
                            lg1[:, q * 512:(q + 1) * 512], cdb,
                            basis_sb[NCC:2 * NCC, 1024 + q * 512: 1024 + (q + 1) * 512],
                            start=True, stop=True,
                        )
                    if b == 0 and sc < 4:
                        # keep the PE busy while the first exps drain the
                        # logit tiles -- a >3.4us PE-idle gap here drops the
                        # HAM clock gate back to 1.2 GHz for ~10us
                        for _ in range(5):
                            nc.tensor.matmul(
                                vt_ps[:, 0:512], junk[:, 0:128],
                                junk[:, 0:512], start=True, stop=True,
                                skip_group_check=True,
                            )
                    g, j = divmod(sc, GRP)
                    if j == 0:
                        sig_tiles[g] = s_pool.tile([128, 2 * GRP], F32, tag="sig",
                                                    name=f"sig{b}_{g}")
                    sig = sig_tiles[g]
                    wt0 = w_pool.tile([128, 1024], BF16, tag="wt")
                    nc.scalar.activation(
                        wt0[:], lg0[:], mybir.ActivationFunctionType.Exp,
                        accum_out=sig[:, j:j + 1],
                    )
                    wt1 = w_pool.tile([128, 1024], BF16, tag="wt")
                    nc.vector._custom_dve(
                        exp_op, out=wt1[:], in0=lg1[:],
                        s0=EXP_C[0], s1=EXP_C[1], imm2=EXP_C[2],
                        accum_out=sig[:, GRP + j:GRP + j + 1],
                    )
                    wts[sc] = (wt0, wt1)
                    if j == GRP - 1:
                        emit_group(g)

                def drain_b(b=b, vt_ps=vt_ps):
                    # drain Vt: split across ScalarE/VectorE (different PSUM
                    # banks -> parallel), then store the l-halves on the two
                    # idle DMA queues
                    out_sb = out_pool.tile([128, 1024], F32, tag="out",
                                           name=f"out{b}")
                    nc.scalar.copy(out_sb[:, 0:512], vt_ps[:, 0:512])
                    nc.vector.tensor_copy(out_sb[:, 512:1024],
                                          vt_ps[:, 512:1024])
                    nc.sync.dma_start(out=out_d[b][:, 0:1024],
                                      in_=out_sb[0:D, :])
                    nc.gpsimd.dma_start(out=out_d[b][:, 1024:2048],
                                        in_=out_sb[D:2 * D, :])

                tail = [(lambda sc=sc, f=emit_acc: f(sc))
                        for sc in range(SC - ACC_LAG, SC)] + [drain_b]
                if b < B - 1:
                    carry_work = tail
                else:
                    for fn in tail:
                        fn()

    nc.compile()
    return nc


def _get_compiled():
    global _COMPILED
    if _COMPILED is None:
        _COMPILED = _build()
    return _COMPILED


def kernel(queries, keys, values):
    global LAST_RESULT
    queries = np.asarray(queries, dtype=np.float32)
    keys = np.asarray(keys, dtype=np.float32)
    values = np.asarray(values, dtype=np.float32)
    bf = ml_dtypes.bfloat16

    # ---- Host coefficient phase (fp32): spectra, cross products,
    # compression, head-mean subtraction.  <1% of total FLOPs. ----
    fx, fy, basis64, mcomp = _constants_f32()
    QT = np.ascontiguousarray(
        queries.reshape(B, L, H, D).transpose(2, 0, 3, 1))   # [H,B,D,L]
    KT = np.ascontiguousarray(
        keys.reshape(B, L, H, D).transpose(2, 0, 3, 1))
    xt = np.matmul(fx.T[None, None], QT)                     # [H,B,128,L]
    yt = np.matmul(fy.T[None, None], KT)
    cf = xt * yt
    ccs = np.matmul(mcomp.T[None, None], cf)                 # [H,B,64,L]
    cd = (ccs - ccs.mean(axis=0, keepdims=True)).astype(bf)  # [H,B,64,L]

    v_swz = np.ascontiguousarray(
        values.reshape(B, SC, 128, H, D).transpose(3, 0, 2, 1, 4)
    ).astype(bf)                                             # [H,B,128,SC,D]
    basis_bf = basis64.astype(bf)

    in_maps = [
        {"cd": np.ascontiguousarray(cd[i]),
         "v": v_swz[i],
         "basis": basis_bf}
        for i in range(NCORES)
    ]

    kw = {"trace_cores": list(range(NCORES))} if TRACE else {}
    cores = list(range(NCORES))
    nc = _get_compiled()
    res = run_bass_kernel_spmd(nc, in_maps, core_ids=cores, trace=TRACE, **kw)
    LAST_RESULT = res

    vt_full = np.stack([res.results[i]["out"] for i in range(NCORES)], axis=1)
    # reference: out = transpose(Vt[B,H,d,L], (0,2,1,3)).reshape(B, L, H*d)
    return np.ascontiguousarray(
        vt_full.transpose(0, 2, 1, 3).reshape(B, L, E)
    ).astype(np.float32)


# revision 30
# speedup vs baseline: 1.2467x; 1.2112x over previous
"""AutoCorrelation (Autoformer-style) Bass kernel for one TRN2 chip (8 NeuronCores).

Math: the reference computes, per (b, h):
    corr = irfft(rfft(q, axis=-1) * conj(rfft(k, axis=-1)), n=L)   # [L, L]
    weights = softmax(corr - mean_h(corr), axis=-1)
    Vt = v @ weights                                                # [d, L]
The rfft runs over the d=64 channel axis and the irfft zero-pads 33 bins to
L=2048, so corr[s, :] is a rank-<=66 function of l; the DC term is constant
over l and cancels in softmax.  Collapsing the spectral products
(re*re + im*im -> cos row, im*re - re*im -> -sin row) leaves 64 coefficient
rows: the logits are an exact K=64 matmul against a fixed cos/sin basis and
no [L, L] tensor ever exists in DRAM.

Sharding: head h -> core h (both batches per core).  The 64 x 2048
coefficient matrices (0.5 MB/core) are computed on the HOST in fp32 --
they are <1% of the FLOPs, and the head-mean that couples cores becomes a
trivial host reduction (an on-device AllReduce costs 55-60 us of fixed
latency on this platform).  The device runs ONE NEFF: the [L, L]-shaped
softmax + delay aggregation, which is >99% of the compute.

Device kernel structure per (b, s-chunk of 128 rows):
  - logits: two row-packed K=64 PE matmul pairs (cd duplicated to partition
    halves 0/64 of both cd and basis) -> two [128, 1024] fp32 PSUM tiles
  - exp: ScalarE table-exp on tile 0, VectorE custom DVE op EXP8_ANT
    (exp(x) ~= (c0 + x(c1 + x c2))^8, valid for |logit| <= ~1.7) on tile 1,
    both writing bf16 weights to SBUF with fused free-dim accumulation for
    the softmax denominator
  - aggregation: column-packed K=128 matmul pairs accumulating Vt into a
    single [128, 1024] fp32 PSUM tile (partitions 0-63 = l 0:1024,
    64-127 = l 1024:2048); the per-row 1/sum folds into the tiny v tile
  - normalization chores (denominator add, reciprocal, v-row scaling) are
    batched 4 s-chunks at a time so GpSimd's ~1us/instruction overhead and
    DVE's per-op overhead amortize; the aggregation lags 6 chunks behind.
PSUM: 3 rotating [128,1024] logit tiles (6 banks) + Vt accumulator (2).
A short burst of dummy matmuls during the DMA preamble warms the PE HAM
clock gate so real matmuls run at 2.4 GHz from the start.
"""
import sys
from operator import add as _op_add

sys.path.insert(0, "/opt/trn_rl_repo")

import numpy as np
import ml_dtypes

from concourse import bass, bacc, mybir, tile
from concourse import dve_ops
from concourse.dve_spec import Spec, Src0, C0, C1, C2, Zero, sq, lower
from concourse.dve_uop import DveOpSpec
from concourse.bass_utils import run_bass_kernel_spmd

B, L, E, H, D = 2, 2048, 512, 8, 64
NF = 32          # frequencies 1..32 of the 64-point rfft (DC dropped)
NCOMP = 4 * NF   # 128 raw product rows
NCC = 2 * NF     # 64 compressed coefficient rows (cos, sin)
NCORES = 8
SC = L // 128    # 16 s-chunks of 128 rows
GRP = 4          # s-chunks per normalization batch
ACC_LAG = 6      # aggregation runs this many s-chunks behind the logits
WARM_MM = 13     # dummy matmuls to warm the PE clock gate during DMA
BF16 = mybir.dt.bfloat16
F32 = mybir.dt.float32

# minimax quadratic p(z) for e^z on z = x/8, |x| <= 1.68; exp(x) ~= p(x)^8
EXP_C = (0.99970171, 0.12580122, 0.00795605)

TRACE = False
LAST_RESULT = None
LAST_RESULT_A = None

_COMPILED = None
_EXP_OP = None


def _register_exp_op():
    global _EXP_OP
    if _EXP_OP is not None:
        return _EXP_OP
    for o in dve_ops.OPS:
        if o.name == "EXP8_ANT":
            _EXP_OP = o
            return o

    body = sq(sq(sq(C0 + Src0 * (C1 + Src0 * C2))))

    def _ref(in0, in1, c0, c1, c2):
        x = in0.astype(np.float32)
        b = (((c0 + x * (c1 + x * c2)) ** 8)).astype(np.float32)
        return b, b.reshape(b.shape[0], -1).sum(axis=-1, keepdims=True)

    spec = Spec(body=body, accum=_op_add, accum_init=Zero, reference=_ref)
    opcode = dve_ops._CUSTOM_DVE_ROW_BASE + len(dve_ops.OPS)
    dve_ops._SUB_OPCODE_FOR_NAME["EXP8_ANT"] = opcode
    shas = {}
    for ver in ("v3", "v4"):
        shas[ver] = DveOpSpec(
            name="EXP8_ANT", opcode=opcode, uops=lower(spec, ver=ver), rd1_en=False
        ).sha(ver)
    op = dve_ops.DveOp("EXP8_ANT", spec, subdim=False, uops_sha=shas)
    dve_ops.OPS.append(op)
    dve_ops.CUSTOM_DVE_SPECS[op.name] = spec
    _EXP_OP = op
    return op


def _constants_f32():
    c = np.arange(D)
    f = np.arange(1, NF + 1)
    ang = 2 * np.pi * np.outer(c, f) / D
    fcos = np.cos(ang)       # Re X_f   = sum_c q_c cos
    fsin = -np.sin(ang)      # Im X_f   = -sum_c q_c sin
    w = 2.0 / L              # irfft weight for interior bins
    fx = np.concatenate([fcos * w, fsin * w, fsin * w, fcos * w], axis=1)  # [64, 128]
    fy = np.concatenate([fcos, fsin, fcos, fsin], axis=1)                  # [64, 128]
    t = np.arange(L)
    angt = 2 * np.pi * np.outer(f, t) / L
    cosb, sinb = np.cos(angt), np.sin(angt)
    basis64 = np.concatenate([cosb, -sinb], axis=0)                        # [64, 2048]
    # compression: Ccs[0:32] = P[0:32] + P[32:64]  (re*re + im*im -> cos)
    #              Ccs[32:64] = P[64:96] - P[96:128] (im*re - re*im -> -sin)
    mcomp = np.zeros((NCOMP, NCC), np.float32)
    for m in range(32):
        mcomp[m, m] = 1.0
        mcomp[m + 32, m] = 1.0
        mcomp[m + 64, m + 32] = 1.0
        mcomp[m + 96, m + 32] = -1.0
    return (fx.astype(np.float32), fy.astype(np.float32),
            basis64.astype(np.float32), mcomp)


def _build():
    exp_op = _register_exp_op()
    nc = bacc.Bacc("TRN2", target_bir_lowering=False, debug=False, num_devices=NCORES)

    cd_d = nc.dram_tensor("cd", [B, NCC, L], BF16, kind="ExternalInput")
    v_d = nc.dram_tensor("v", [B, 128, SC, D], BF16, kind="ExternalInput")
    basis_d = nc.dram_tensor("basis", [NCC, L], BF16, kind="ExternalInput")
    out_d = nc.dram_tensor("out", [B, D, L], F32, kind="ExternalOutput")

    with tile.TileContext(nc) as tc:
        with (
            tc.tile_pool(name="consts", bufs=1) as consts,
            tc.tile_pool(name="vv", bufs=2) as v_pool,
            tc.tile_pool(name="cd", bufs=2) as cd_pool,
            tc.tile_pool(name="wts", bufs=24) as w_pool,
            tc.tile_pool(name="small", bufs=12) as s_pool,
            tc.tile_pool(name="outp", bufs=2) as out_pool,
            tc.tile_pool(name="ps_log", bufs=3, space="PSUM") as ps_log,
            tc.tile_pool(name="ps_vt", bufs=1, space="PSUM") as ps_vt,
        ):
            # ---- DMA preamble: dedup duplicated partition halves, spread
            # across queues, order so b=0's operands land first. ----
            basis_sb = consts.tile([NCOMP, L], BF16)
            nc.sync.dma_start(out=basis_sb[0:NCC, :], in_=basis_d[:])
            nc.sync.dma_start(out=basis_sb[NCC:NCOMP, :], in_=basis_d[:])
            cd_sbs = []
            for b in range(B):
                cdd = cd_pool.tile([2 * NCC, L], BF16, tag=f"cd{b}")
                eng = nc.scalar
                eng.dma_start(out=cdd[0:NCC, :], in_=cd_d[b])
                eng.dma_start(out=cdd[NCC:2 * NCC, :], in_=cd_d[b])
                cd_sbs.append(cdd)
            v_sbs = []
            for b in range(B):
                v_sb = v_pool.tile([128, SC, D], BF16, tag=f"v{b}")
                nc.gpsimd.dma_start(out=v_sb[:], in_=v_d[b])
                v_sbs.append(v_sb)

            # ---- PE warmup: junk matmuls (no DMA deps) flip the HAM clock
            # gate to 2.4 GHz while the inputs stream in. ----
            junk = consts.tile([128, 512], BF16)
            nc.vector.memset(junk[:], 0.0)
            warm_ps = ps_log.tile([128, 1024], F32, tag="log")
            for _ in range(WARM_MM):
                nc.tensor.matmul(
                    warm_ps[:, 0:512], junk[:, 0:128], junk[:, 0:512],
                    start=True, stop=True,
                )

            carry_work = []
            for b in range(B):
                cdd = cd_sbs[b]
                v_sb = v_sbs[b]
                vt_ps = ps_vt.tile([128, 1024], F32, tag="vt")

                wts = {}
                vts_tiles = {}
                sig_tiles = {}

                def emit_acc(sc, wts=wts, vts_tiles=vts_tiles,
                             vt_ps=vt_ps):
                    wt0, wt1 = wts.pop(sc)
                    g, j = divmod(sc, GRP)
                    vts = vts_tiles[g]
                    lhs = vts[:, j, :]
                    for q in range(2):
                        # column-packed pair: l-half 0 on PE cols 0-63,
                        # l-half 1 on cols 64-127, concurrent
                        nc.tensor.matmul(
                            vt_ps[0:D, q * 512:(q + 1) * 512],
                            lhs, wt0[:, q * 512:(q + 1) * 512],
                            start=(sc == 0), stop=(sc == SC - 1),
                        )
                        nc.tensor.matmul(
                            vt_ps[D:2 * D, q * 512:(q + 1) * 512],
                            lhs, wt1[:, q * 512:(q + 1) * 512],
                            start=(sc == 0), stop=(sc == SC - 1),
                        )
                    if j == GRP - 1:
                        vts_tiles.pop(g)

                def emit_group(g):
                    # denominators for s-chunks 4g..4g+3 are complete:
                    # Z = sigS + sigV, rcp = 1/Z, vts = v * rcp -- one
                    # batched instruction each.
                    sig = sig_tiles.pop(g)
                    zs = s_pool.tile([128, GRP], F32, tag="zs", name=f"zs{b}_{g}")
                    nc.gpsimd.tensor_add(zs[:], sig[:, 0:GRP], sig[:, GRP:2 * GRP])
                    rcp = s_pool.tile([128, GRP], F32, tag="rcp", name=f"rcp{b}_{g}")
                    nc.vector.reciprocal_approx_fast(rcp[:], zs[:])
                    vts = s_pool.tile([128, GRP, D], BF16, tag="vts", name=f"vts{b}_{g}")
                    rcp_b = rcp[:].broadcast_to([128, GRP, D])
                    nc.gpsimd.tensor_mul(vts[:], v_sb[:, GRP * g:GRP * (g + 1), :],
                                         rcp_b)
                    vts_tiles[g] = vts

                for sc in range(SC):
                    off = sc * 128
                    cdt = cdd[0:NCC, off:off + 128]
                    cdb = cdd[NCC:2 * NCC, off:off + 128]
                    # aggregation first: it is always ready (lags 5
                    # chunks), so the in-order PE never head-of-line blocks
                    # on a logit-tile wait while ready work sits behind it
                    if sc >= ACC_LAG:
                        emit_acc(sc - ACC_LAG)
                    # drain the previous batch's carried-over aggregation
                    # flush + Vt drain into our early iterations, so the PE
                    # and DMA queues never go quiet at the batch boundary
                    for _ in range(2):
                        if carry_work:
                            carry_work.pop(0)()
                    # allocate lg1 first: with the 3-buffer rotation the
                    # FIRST allocation reuses the tile freed longest ago.
                    # lg1 feeds VectorE (1.31us/tile), lg0 feeds ScalarE
                    # (1.25us) -- the slower consumer should get the
                    # longer-lag tile, not the other way around.
                    lg1 = ps_log.tile([128, 1024], F32, tag="log",
                                      name="lg1")
                    lg0 = ps_log.tile([128, 1024], F32, tag="log",
                                      name="lg0")
                    for q in range(2):
                        # row-packed pair: l-half 0 on PE rows 0-63,
                        # l-half 1 on rows 64-127, concurrent
                        nc.tensor.matmul(
                            lg0[:, q * 512:(q + 1) * 512], cdt,
                            basis_sb[0:NCC, q * 512:(q + 1) * 512],
                            start=True, stop=True,
                        )
                        nc.tensor.matmul(
                            lg1[:, q * 512:(q + 1) * 512], cdb,
                            basis_sb[NCC:2 * NCC, 1024 + q * 512: 1024 + (q + 1) * 512],
                            start=True, stop=True,
                        )
                    if b == 0 and sc < 4:
                        # keep the PE busy while the first exps drain the
                        # logit tiles -- a >3.4us PE-idle gap here drops the
                        # HAM clock gate back to 1.2 GHz for ~10us
                        for _ in range(5):
                            nc.tensor.matmul(
                                vt_ps[:, 0:512], junk[:, 0:128],
                                junk[:, 0:512], start=True, stop=True,
                                skip_group_check=True,
                            )
                    g, j = divmod(sc, GRP)
                    if j == 0:
                        sig_tiles[g] = s_pool.tile([128, 2 * GRP], F32, tag="sig",
                                                    name=f"sig{b}_{g}")
                    sig = sig_tiles[g]
                    wt0 = w_pool.tile([128, 1024], BF16, tag="wt")
                    nc.scalar.activation(
                        wt0[:], lg0[:], mybir.ActivationFunctionType.Exp,
                        accum_out=sig[:, j:j + 1],
                    )
                    wt1 = w_pool.tile([128, 1024], BF16, tag="wt")
                    nc.vector._custom_dve(
                        exp_op, out=wt1[:], in0=lg1[:],
                        s0=EXP_C[0], s1=EXP_C[1], imm2=EXP_C[2],
                        accum_out=sig[:, GRP + j:GRP + j + 1],
                    )
                    wts[sc] = (wt0, wt1)
                    if j == GRP - 1:
                        emit_group(g)

                def drain_b(b=b, vt_ps=vt_ps):
                    # drain Vt: split across ScalarE/VectorE (different PSUM
                    # banks -> parallel), then store the l-halves on the two
                    # idle DMA queues
                    out_sb = out_pool.tile([128, 1024], F32, tag="out",
                                           name=f"out{b}")
                    nc.scalar.copy(out_sb[:, 0:512], vt_ps[:, 0:512])
                    nc.vector.tensor_copy(out_sb[:, 512:1024],
                                          vt_ps[:, 512:1024])
                    nc.sync.dma_start(out=out_d[b][:, 0:1024],
                                      in_=out_sb[0:D, :])
                    nc.gpsimd.dma_start(out=out_d[b][:, 1024:2048],
                                        in_=out_sb[D:2 * D, :])

                tail = [(lambda sc=sc, f=emit_acc: f(sc))
                        for sc in range(SC - ACC_LAG, SC)] + [drain_b]
                if b < B - 1:
                    carry_work = tail
                else:
                    for fn in tail:
                        fn()

    nc.compile()
    return nc


def _get_compiled():
    global _COMPILED
    if _COMPILED is None:
        _COMPILED = _build()
    return _COMPILED


def kernel(queries, keys, values):
    global LAST_RESULT
    queries = np.asarray(queries, dtype=np.float32)
    keys = np.asarray(keys, dtype=np.float32)
    values = np.asarray(values, dtype=np.float32)
    bf = ml_dtypes.bfloat16

    # ---- Host coefficient phase (fp32): spectra, cross products,
    # compression, head-mean subtraction.  <1% of total FLOPs. ----
    fx, fy, basis64, mcomp = _constants_f32()
    QT = np.ascontiguousarray(
        queries.reshape(B, L, H, D).transpose(2, 0, 3, 1))   # [H,B,D,L]
    KT = np.ascontiguousarray(
        keys.reshape(B, L, H, D).transpose(2, 0, 3, 1))
    xt = np.matmul(fx.T[None, None], QT)                     # [H,B,128,L]
    yt = np.matmul(fy.T[None, None], KT)
    cf = xt * yt
    ccs = np.matmul(mcomp.T[None, None], cf)                 # [H,B,64,L]
    cd = (ccs - ccs.mean(axis=0, keepdims=True)).astype(bf)  # [H,B,64,L]

    v_swz = np.ascontiguousarray(
        values.reshape(B, SC, 128, H, D).transpose(3, 0, 2, 1, 4)
    ).astype(bf)                                             # [H,B,128,SC,D]
    basis_bf = basis64.astype(bf)

    in_maps = [
        {"cd": np.ascontiguousarray(cd[i]),
         "v": v_swz[i],
         "basis": basis_bf}
        for i in range(NCORES)
    ]

    kw = {"trace_cores": list(range(NCORES))} if TRACE else {}
    cores = list(range(NCORES))
    nc = _get_compiled()
    res = run_bass_kernel_spmd(nc, in_maps, core_ids=cores, trace=TRACE, **kw)
    LAST_RESULT = res

    vt_full = np.stack([res.results[i]["out"] for i in range(NCORES)], axis=1)
    # reference: out = transpose(Vt[B,H,d,L], (0,2,1,3)).reshape(B, L, H*d)
    return np.ascontiguousarray(
        vt_full.transpose(0, 2, 1, 3).reshape(B, L, E)
    ).astype(np.float32)
